# revision 1
# baseline (speedup 1.0000x reference)
"""AGNN (3-layer cosine-attention message passing) on 8 trn2 NeuronCores.

Self-contained: host-side graph prep (numpy) + Bass/Tile device program +
run via run_bass_kernel_spmd. kernel(**inputs) takes the full unsharded
inputs and returns the full [G, C] output.

Sharding: nodes (and their incoming edges) are partitioned across the 8
cores by dst; each core holds a replicated node-feature table in DRAM
([nh*sqrt(beta) | h | 1] fp16 rows), gathers src rows / dst nh-halves with
indirect DMA, does the edge softmax + scatter as masked matmuls into PSUM,
and an AllGather rebuilds the replicated table between layers.  The final
graph readout (mean-pool partials) is summed on host and put through the
tiny [64,128]@[128,100] classifier in numpy.
"""

import sys

sys.path.insert(0, "/opt/trn_rl_repo")

import numpy as np

import concourse.bass as bass
import concourse.bacc as bacc
import concourse.mybir as mybir
import concourse.tile as tile

EPS = 1e-12


# ---------------------------------------------------------------- config

class Cfg:
    def __init__(self, N, E, G, NC, blocks_per_sb=3):
        self.N = N                    # real nodes
        self.E = E                    # edges
        self.G = G                    # graphs
        self.NC = NC                  # cores
        self.D = 128
        self.NPC = N // NC            # real nodes per core
        self.BLK = 128
        self.NBLK = -(-self.NPC // self.BLK)       # blocks per core
        self.NPAD = self.NBLK * self.BLK           # padded nodes per core
        self.NPADTOT = self.NPAD * NC
        self.ROW = 258                # [nh 128 | h 128 | 1 | pad]
        self.SBS = blocks_per_sb      # dst blocks per super-block
        self.NSB = -(-self.NBLK // self.SBS)
        self.L = 3


# ---------------------------------------------------------------- host prep

def _prep(cfg, h, src, dst, graph_ids, betas):
    """Build per-core input maps + the shared tile schedule."""
    N, NC, NPC, NPAD, BLK, NBLK = cfg.N, cfg.NC, cfg.NPC, cfg.NPAD, cfg.BLK, cfg.NBLK
    h = np.asarray(h, np.float32)
    src = np.asarray(src, np.int64)
    dst = np.asarray(dst, np.int64)
    graph_ids = np.asarray(graph_ids, np.int64)
    betas = np.asarray(betas, np.float32)

    norms = np.sqrt((h * h).sum(1))
    nh = h / (norms + EPS)[:, None]

    tab0 = np.zeros((cfg.NPADTOT, cfg.ROW), np.float16)
    sb0 = np.sqrt(betas[0])
    for c in range(NC):
        r0 = c * NPAD
        nr = slice(c * NPC, (c + 1) * NPC)
        tab0[r0:r0 + NPC, 0:128] = (nh[nr] * sb0).astype(np.float16)
        tab0[r0:r0 + NPC, 128:256] = h[nr].astype(np.float16)
        tab0[r0:r0 + NPC, 256] = 1.0

    # edges sorted by global dst -> grouped by (core, local block)
    order = np.argsort(dst, kind="stable")
    e_src = src[order]
    e_dst = dst[order]
    src_pad = (e_src // NPC) * NPAD + (e_src % NPC)
    dcore = e_dst // NPC
    dlocal = e_dst % NPC
    dblk = dlocal // BLK

    # per (core, block) edge counts -> shared tile schedule
    cnt = np.zeros((NC, NBLK), np.int64)
    np.add.at(cnt, (dcore, dblk), 1)
    T_b = np.maximum(1, -(-cnt.max(0) // 128))     # tiles per block (shared)
    Ttot = int(T_b.sum())
    tile_block = np.repeat(np.arange(NBLK), T_b)   # block of each tile col
    tcol0 = np.zeros(NBLK, np.int64)               # first tile col per block
    tcol0[1:] = np.cumsum(T_b)[:-1]

    in_maps = []
    sqbeta = np.zeros((128, 4), np.float32)
    sqbeta[:, 0] = np.sqrt(betas[1]) if len(betas) > 1 else 1.0
    sqbeta[:, 1] = np.sqrt(betas[2]) if len(betas) > 2 else 1.0

    for c in range(NC):
        dummy = c * NPAD + min(NPC, NPAD - 1)
        srcidx = np.full((128, Ttot), dummy, np.int32)
        dstrel = np.full((128, Ttot), -1.0, np.float16)

        m = dcore == c
        cs, cl, cb = src_pad[m], dlocal[m], dblk[m]
        for b in range(NBLK):
            bm = cb == b
            bs = cs[bm]
            br = (cl[bm] - b * BLK).astype(np.float16)
            n = len(bs)
            t0 = tcol0[b]
            for t in range(T_b[b]):
                lo, hi = t * 128, min((t + 1) * 128, n)
                if lo >= hi:
                    break
                srcidx[0:hi - lo, t0 + t] = bs[lo:hi]
                dstrel[0:hi - lo, t0 + t] = br[lo:hi]
        # transposed-broadcast copy of dstrel: plane t row p = dstrel[:, t]
        dstrelT = np.broadcast_to(
            dstrel.T.reshape(1, Ttot * 128).astype(np.int8),
            (128, Ttot * 128)).copy()

        selg = np.zeros((128, NBLK * cfg.G), np.float16)
        gid = graph_ids[c * NPC:(c + 1) * NPC]
        for b in range(NBLK):
            p = np.arange(b * BLK, min((b + 1) * BLK, NPC))
            selg[p - b * BLK, b * cfg.G + gid[p]] = 1.0

        myrows0 = tab0[c * NPAD:(c + 1) * NPAD].copy()
        in_maps.append(dict(
            tab0=tab0, srcidx=srcidx, dstrel=dstrel, dstrelT=dstrelT,
            myrows0=myrows0, selg=selg, sqbeta=sqbeta,
        ))

    counts = np.bincount(graph_ids, minlength=cfg.G).astype(np.float32)
    sched = dict(T_b=[int(x) for x in T_b], tcol0=[int(x) for x in tcol0],
                 tile_block=[int(x) for x in tile_block], Ttot=Ttot)
    return in_maps, counts, sched


# ---------------------------------------------------------------- device program

def build_program(cfg, sched, trace_sim=False, debug=False):
    f16, f32, i32 = mybir.dt.float16, mybir.dt.float32, mybir.dt.int32
    T_b, tcol0 = sched["T_b"], sched["tcol0"]
    Ttot = sched["Ttot"]
    NBLK, SBS, NSB, ROW, G = cfg.NBLK, cfg.SBS, cfg.NSB, cfg.ROW, cfg.G
    Tmax = max(sum(T_b[sb * SBS:(sb + 1) * SBS]) for sb in range(NSB))

    nc = bacc.Bacc("TRN2", target_bir_lowering=False, debug=False,
                   num_devices=cfg.NC)

    tab0 = nc.dram_tensor("tab0", [cfg.NPADTOT, ROW], f16, kind="ExternalInput").ap()
    srcidx = nc.dram_tensor("srcidx", [128, Ttot], i32, kind="ExternalInput").ap()
    dstrel = nc.dram_tensor("dstrel", [128, Ttot], f16, kind="ExternalInput").ap()
    dstrelT = nc.dram_tensor("dstrelT", [128, Ttot * 128], mybir.dt.int8,
                             kind="ExternalInput").ap()
    myrows0 = nc.dram_tensor("myrows0", [cfg.NPAD, ROW], f16,
                             kind="ExternalInput").ap()
    selg_d = nc.dram_tensor("selg", [128, NBLK * G], f16, kind="ExternalInput").ap()
    sqbeta_d = nc.dram_tensor("sqbeta", [128, 4], f32, kind="ExternalInput").ap()
    pooled_d = nc.dram_tensor("pooled", [G, 128], f32, kind="ExternalOutput").ap()

    shard = [nc.dram_tensor(f"shard{l}", [cfg.NPAD, ROW], f16).ap()
             for l in range(cfg.L - 1)]
    localrows = [myrows0] + shard          # per-layer local (own-node) rows
    tab_space = "Shared" if cfg.NC > 4 else "Local"
    tabs = [tab0] + [
        nc.dram_tensor(f"tab{l + 1}", [cfg.NPADTOT, ROW], f16,
                       addr_space=tab_space).ap()
        for l in range(cfg.L - 1)]

    groups = [list(range(cfg.NC))]

    from contextlib import ExitStack

    with tile.TileContext(nc, trace_sim=trace_sim) as tc, ExitStack() as ctx:
        const = ctx.enter_context(tc.tile_pool(name="const", bufs=1))
        iota_i = const.tile([128, 128], i32)
        nc.gpsimd.iota(iota_i[:], pattern=[[1, 128]], base=0, channel_multiplier=0)
        iota_f = const.tile([128, 128], f16)
        nc.vector.tensor_copy(iota_f[:], iota_i[:])
        iotac_i = const.tile([128, 1], i32)
        nc.gpsimd.iota(iotac_i[:], pattern=[[0, 1]], base=0, channel_multiplier=1)
        iotac_f = const.tile([128, 1], f16)
        nc.vector.tensor_copy(iotac_f[:], iotac_i[:])
        selg_s = const.tile([128, NBLK * G], f16)
        nc.sync.dma_start(selg_s[:], selg_d)
        sqbeta = const.tile([128, 4], f32)
        nc.sync.dma_start(sqbeta[:], sqbeta_d)

        idxp = ctx.enter_context(tc.tile_pool(name="idxp", bufs=3))
        gp = ctx.enter_context(tc.tile_pool(name="gp", bufs=2))
        cp = ctx.enter_context(tc.tile_pool(name="cp", bufs=2))
        ep = ctx.enter_context(tc.tile_pool(name="ep", bufs=2))
        pp = ctx.enter_context(tc.tile_pool(name="pp", bufs=2, space="PSUM"))
        pp2 = ctx.enter_context(tc.tile_pool(name="pp2", bufs=2, space="PSUM"))
        ppool = ctx.enter_context(tc.tile_pool(name="ppool", bufs=1, space="PSUM"))

        pool_ps = ppool.tile([G, 128], f32, tag="pool")

        for l in range(cfg.L):
            tab = tabs[l]
            for sb in range(NSB):
                blocks = list(range(sb * SBS, min((sb + 1) * SBS, NBLK)))
                nb = len(blocks)
                c0 = tcol0[blocks[0]]
                Tsb = sum(T_b[b] for b in blocks)

                tile_bi = []
                for bi, b in enumerate(blocks):
                    tile_bi += [bi] * T_b[b]

                # ---- indices / masks / local dst rows
                idx_s = idxp.tile([128, Tmax], i32, tag="idxs")
                nc.sync.dma_start(idx_s[:, 0:Tsb], srcidx[:, c0:c0 + Tsb])
                drel = idxp.tile([128, Tmax], f16, tag="drel")
                nc.sync.dma_start(drel[:, 0:Tsb], dstrel[:, c0:c0 + Tsb])
                drelT = gp.tile([128, Tmax * 128], mybir.dt.int8, tag="drelT")
                nc.sync.dma_start(drelT[:, 0:Tsb * 128],
                                  dstrelT[:, c0 * 128:(c0 + Tsb) * 128])
                nhblk = idxp.tile([128, SBS * 128], f16, tag="nhblk")
                nb3 = nhblk[:, 0:nb * 128].rearrange("p (b d) -> p b d", d=128)
                nc.sync.dma_start(
                    nb3,
                    localrows[l][sb * SBS * 128: sb * SBS * 128 + nb * 128, 0:128]
                    .rearrange("(b p) d -> p b d", p=128))

                # ---- src gather: one [128,1]-offset call per 128-edge tile
                # (HW contract: partition p reads a contiguous line from
                # row idx[p]; multi-column offset APs are NOT honored)
                gsrc = gp.tile([128, Tmax * ROW], f16, tag="gsrc")
                for t in range(Tsb):
                    nc.gpsimd.indirect_dma_start(
                        out=gsrc[:, t * ROW:(t + 1) * ROW], out_offset=None,
                        in_=tab, in_offset=bass.IndirectOffsetOnAxis(
                            ap=idx_s[:, t:t + 1], axis=0))
                g3 = gsrc[:, 0:Tsb * ROW].rearrange("p (t d) -> p t d", d=ROW)

                # ---- dst expansion on PE:  selT[j, e] = (j == dstrel[e])
                selT = gp.tile([128, Tmax * 128], f16, tag="selT")
                sT3 = selT[:, 0:Tsb * 128].rearrange("p (t j) -> p t j", j=128)
                ioc_b = iotac_f[:].rearrange("p (a b) -> p a b", a=1) \
                    .to_broadcast([128, Tsb, 128])
                dT3 = drelT[:, 0:Tsb * 128].rearrange("p (t j) -> p t j", j=128)
                nc.vector.tensor_tensor(
                    out=sT3, in0=ioc_b, in1=dT3, op=mybir.AluOpType.is_equal)

                # ---- scores in sub-groups of 4 tiles (one PSUM bank each)
                s_t = cp.tile([128, Tmax], f32, tag="s")
                for g0 in range(0, Tsb, 8):
                    gn = min(8, Tsb - g0)
                    px = pp2.tile([128, 1024], f32, tag="px")
                    for k in range(gn):
                        t = g0 + k
                        nc.tensor.matmul(
                            out=px[:, k * 128:(k + 1) * 128],
                            lhsT=sT3[:, t, :], rhs=nb3[:, tile_bi[t], :],
                            start=True, stop=True)
                    prod = cp.tile([128, 1024], f16, tag="prod")
                    p3 = prod[:, 0:gn * 128].rearrange("p (t d) -> p t d", d=128)
                    nc.vector.tensor_tensor(
                        out=p3, in0=px[:, 0:gn * 128].rearrange(
                            "p (t d) -> p t d", d=128),
                        in1=g3[:, g0:g0 + gn, 0:128], op=mybir.AluOpType.mult)
                    nc.vector.tensor_reduce(
                        out=s_t[:, g0:g0 + gn], in_=p3,
                        axis=mybir.AxisListType.X, op=mybir.AluOpType.add)
                a_t = cp.tile([128, Tmax], f16, tag="a")
                nc.scalar.activation(
                    out=a_t[:, 0:Tsb], in_=s_t[:, 0:Tsb],
                    func=mybir.ActivationFunctionType.Exp)

                # ---- masked attention:  asel[e, j] = a[e] * (iota[j] == dstrel[e])
                sel = gp.tile([128, Tmax * 128], f16, tag="sel")
                s3 = sel[:, 0:Tsb * 128].rearrange("p (t j) -> p t j", j=128)
                io_b = iota_f[:].rearrange("p (o j) -> p o j", o=1) \
                    .to_broadcast([128, Tsb, 128])
                dr_b = drel[:, 0:Tsb].rearrange("p (t o) -> p t o", o=1) \
                    .to_broadcast([128, Tsb, 128])
                nc.vector.tensor_tensor(
                    out=s3, in0=io_b, in1=dr_b, op=mybir.AluOpType.is_equal)
                a_b = a_t[:, 0:Tsb].rearrange("p (t o) -> p t o", o=1) \
                    .to_broadcast([128, Tsb, 128])
                nc.vector.tensor_tensor(
                    out=s3, in0=s3, in1=a_b, op=mybir.AluOpType.mult)

                # ---- scatter:  psum[:, bb*129:(bb+1)*129] += asel_t^T @ [h|1]
                pn = pp.tile([128, 512], f32, tag="pn")
                tt = 0
                for bi, b in enumerate(blocks):
                    for t in range(T_b[b]):
                        nc.tensor.matmul(
                            out=pn[:, bi * 129:bi * 129 + 129],
                            lhsT=s3[:, tt, :],
                            rhs=g3[:, tt, 128:257],
                            start=(t == 0), stop=(t == T_b[b] - 1))
                        tt += 1

                # ---- epilogue: h' = num / max(den, tiny)
                p3 = pn[:, 0:nb * 129].rearrange("p (b d) -> p b d", d=129)
                den = ep.tile([128, SBS], f32, tag="den")
                nc.vector.tensor_scalar_max(den[:, 0:nb], p3[:, :, 128:129], 1e-30)
                rec = ep.tile([128, SBS], f32, tag="rec")
                nc.vector.reciprocal(rec[:, 0:nb], den[:, 0:nb])
                hsb = ep.tile([128, SBS * 128], f32, tag="hsb")
                h3 = hsb[:, 0:nb * 128].rearrange("p (b d) -> p b d", d=128)
                rec_b = rec[:, 0:nb].rearrange("p (b o) -> p b o", o=1) \
                    .to_broadcast([128, nb, 128])
                nc.vector.tensor_tensor(
                    out=h3, in0=p3[:, :, 0:128], in1=rec_b,
                    op=mybir.AluOpType.mult)

                if l < cfg.L - 1:
                    # rebuild table rows [nh*sqrt(beta) | h | 1]
                    sq = ep.tile([128, SBS * 128], f32, tag="sq")
                    q3 = sq[:, 0:nb * 128].rearrange("p (b d) -> p b d", d=128)
                    nc.vector.tensor_tensor(out=q3, in0=h3, in1=h3,
                                            op=mybir.AluOpType.mult)
                    ss = ep.tile([128, SBS], f32, tag="ss")
                    nc.vector.tensor_reduce(
                        out=ss[:, 0:nb], in_=q3, axis=mybir.AxisListType.X,
                        op=mybir.AluOpType.add)
                    nrm = ep.tile([128, SBS], f32, tag="nrm")
                    nc.scalar.sqrt(nrm[:, 0:nb], ss[:, 0:nb])
                    nc.vector.tensor_scalar_add(nrm[:, 0:nb], nrm[:, 0:nb], EPS)
                    rn = ep.tile([128, SBS], f32, tag="rn")
                    nc.vector.reciprocal(rn[:, 0:nb], nrm[:, 0:nb])

                    stg = ep.tile([128, SBS * ROW], f16, tag="stg")
                    st3 = stg[:, 0:nb * ROW].rearrange("p (b d) -> p b d", d=ROW)
                    rn_b = rn[:, 0:nb].rearrange("p (b o) -> p b o", o=1) \
                        .to_broadcast([128, nb, 128])
                    nc.vector.scalar_tensor_tensor(
                        out=st3[:, :, 0:128], in0=h3,
                        scalar=sqbeta[:, l:l + 1], in1=rn_b,
                        op0=mybir.AluOpType.mult, op1=mybir.AluOpType.mult)
                    nc.vector.tensor_copy(out=st3[:, :, 128:256], in_=h3)
                    nc.vector.memset(st3[:, :, 256:258], 1.0)

                    out_ap = shard[l][sb * SBS * 128: sb * SBS * 128 + nb * 128, :] \
                        .rearrange("(b p) d -> p b d", p=128)
                    nc.sync.dma_start(out_ap, st3)
                else:
                    hf = ep.tile([128, SBS * 128], f16, tag="hf")
                    hf3 = hf[:, 0:nb * 128].rearrange("p (b d) -> p b d", d=128)
                    nc.vector.tensor_copy(out=hf3, in_=h3)
                    for bi, b in enumerate(blocks):
                        nc.tensor.matmul(
                            out=pool_ps[:, :],
                            lhsT=selg_s[:, b * G:b * G + G],
                            rhs=hf3[:, bi, :],
                            start=(b == 0), stop=(b == NBLK - 1))

            if l < cfg.L - 1:
                nc.gpsimd.collective_compute(
                    "AllGather", mybir.AluOpType.bypass,
                    replica_groups=groups,
                    ins=[shard[l][:, :]], outs=[tabs[l + 1][:, :]])

        pooled_s = const.tile([G, 128], f32)
        nc.scalar.copy(out=pooled_s[:, :], in_=pool_ps[:, :])
        nc.sync.dma_start(pooled_d, pooled_s[:, :])

        if debug:
            dbg_sh = nc.dram_tensor("dbg_shard0", [cfg.NPAD, ROW], f16,
                                    kind="ExternalOutput").ap()
            nc.sync.dma_start(dbg_sh, shard[0][:, :])
            dbg_tab = nc.dram_tensor("dbg_tab1", [cfg.NPADTOT, ROW], f16,
                                     kind="ExternalOutput").ap()
            nc.sync.dma_start(dbg_tab, tabs[1][:, :])

    return nc


# ---------------------------------------------------------------- entry

LAST_EXEC_NS = None
_CACHE = {}


def _get_compiled(cfg, sched):
    key = tuple(sched["T_b"])
    if key not in _CACHE:
        nc = build_program(cfg, sched)
        nc.compile()
        _CACHE[key] = nc
    return _CACHE[key]


def kernel(h, src, dst, graph_ids, betas, W_cls, b_cls, time_execs=0):
    global LAST_EXEC_NS
    import time as _time
    from concourse.bass_utils import run_bass_kernel_spmd

    cfg = Cfg(N=40000, E=640000, G=64, NC=8)
    in_maps, counts, sched = _prep(cfg, h, src, dst, graph_ids, betas)
    nc = _get_compiled(cfg, sched)

    def _run():
        last = None
        for attempt in range(3):
            try:
                return run_bass_kernel_spmd(nc, in_maps,
                                            core_ids=list(range(cfg.NC)))
            except Exception as e:  # transient axon worker hangs
                last = e
                _time.sleep(5)
        raise last

    res = _run()
    if time_execs:
        # no NTFF profiling hook is available in this container, so report
        # median wall-clock of repeated NEFF executions (includes the axon
        # dispatch overhead; on-device time is lower)
        ts = []
        for _ in range(time_execs):
            t0 = _time.time()
            res = run_bass_kernel_spmd(nc, in_maps, core_ids=list(range(cfg.NC)))
            ts.append(_time.time() - t0)
        LAST_EXEC_NS = int(np.median(ts) * 1e9)
    pooled = np.zeros((cfg.G, 128), np.float64)
    for r in res.results:
        pooled[: , :] += r["pooled"][:cfg.G].astype(np.float64)
    hg = (pooled / np.maximum(counts, 1.0)[:, None]).astype(np.float32)
    return hg @ np.asarray(W_cls, np.float32) + np.asarray(b_cls, np.float32)

